# revision 1
# baseline (speedup 1.0000x reference)
"""Trainium2 Bass kernel for the soft-decision-tree ensemble classifier.

Math (per batch row b, tree t):
  zb[t,n]      = x[b] . W[t,n] + bias[t,n]
  log s        = zb - softplus(zb);  log(1-s) = -softplus(zb)
  log_leaf[l]  = sum_{k in path(l)} dir_k * zb_k  -  sum_{k in path(l)} softplus(zb_k)
  leaf_prob    = exp(log_leaf)
  out[b,c]     = sum_t 2*softmax(tw)_t * sum_l leaf_prob[t,l] * softmax(leaf_logits[t,l])_c

Mapping: data-parallel over the batch (B=4096 -> 512 rows per NeuronCore).
Per core, logits live in [tree-node (padded 64/tree), batch] layout so the
per-tree path sums become 128-wide matmuls with +/-1 constant matrices
(block-diagonal over a pair of trees per 128-partition tile). All matmuls
run in fp16 (1 cycle/row on the PE; ~8x finer mantissa than bf16, which
keeps the end-to-end error ~6e-4). softplus is computed as Ln(Exp(x)+1) so
the whole kernel needs a single ACT function table (pinned up front - the
automatic table chooser would otherwise reload tables between Exp and Ln
constantly, ~1.3us per reload). Exp/Ln are batched 4 tiles per instruction
and the path-sum outputs are paired two-to-a-PSUM-bank-pair so the exp that
follows runs once per pair. Input DMAs are chunked and spread across the
SP and ACT DGE queues so the first matmul can start as early as possible.
"""

import numpy as np

TREE_DEPTH = 6
T, N, D, C = 64, 63, 512, 100
L = 2**TREE_DEPTH          # 64
NPAD = 64                  # nodes padded per tree
TNP = T * NPAD             # 4096
NTILES = TNP // 128        # 32 (two trees per 128-partition tile)
B = 4096
NCORES = 8
BS = B // NCORES           # 512

# column layout of the packed constants tensor [128, 512]
_COL_BIAS = 0      # [128, 32]
_COL_ADIR = 32     # [128, 128]
_COL_APATH = 160   # [128, 128]
_COL_A2 = 288      # [64, 128]
_COL_E2 = 416      # [64, 32]
_COL_TW = 448      # [1, 64]
_CONST_COLS = 512


def _leaf_paths(depth):
    Ll = 2**depth
    idx = np.zeros((Ll, depth), np.int32)
    dr = np.zeros((Ll, depth), np.int32)
    for l in range(Ll):
        node = 0
        for k in range(depth):
            bit = (l >> (depth - 1 - k)) & 1
            idx[l, k] = node
            dr[l, k] = bit
            node = 2 * node + 1 + bit
    return idx, dr


def _pack_consts(split_bias, tree_weights):
    """Build the [128, 512] packed constants array (f32 bits)."""
    idx, dr = _leaf_paths(TREE_DEPTH)
    mdir = np.zeros((NPAD, L), np.float32)   # [node, leaf] +1 where dir=1
    mpath = np.zeros((NPAD, L), np.float32)  # [node, leaf] -1 on path
    for l in range(L):
        for k in range(TREE_DEPTH):
            n = idx[l, k]
            mpath[n, l] -= 1.0
            if dr[l, k]:
                mdir[n, l] += 1.0
    consts = np.zeros((128, _CONST_COLS), np.float32)
    # bias columns: bias_pad flattened [(tile, partition)] -> [128, 32]
    bpad = np.zeros((T, NPAD), np.float32)
    bpad[:, :N] = split_bias
    consts[:, _COL_BIAS:_COL_BIAS + NTILES] = bpad.reshape(NTILES, 128).T
    # block-diagonal path matrices (two trees per 128-tile)
    consts[:NPAD, _COL_ADIR:_COL_ADIR + L] = mdir
    consts[NPAD:, _COL_ADIR + L:_COL_ADIR + 128] = mdir
    consts[:NPAD, _COL_APATH:_COL_APATH + L] = mpath
    consts[NPAD:, _COL_APATH + L:_COL_APATH + 128] = mpath
    # a2[t, p] = 1 if (t % 2) == p // 64 — broadcast selector for w2 columns
    a2 = np.zeros((T, 128), np.float32)
    for t in range(T):
        a2[t, (t % 2) * 64:(t % 2) * 64 + 64] = 1.0
    consts[:T, _COL_A2:_COL_A2 + 128] = a2
    # e2[t, i] = 1 if t // 2 == i
    e2 = np.zeros((T, NTILES), np.float32)
    for t in range(T):
        e2[t, t // 2] = 1.0
    consts[:T, _COL_E2:_COL_E2 + NTILES] = e2
    consts[0, _COL_TW:_COL_TW + T] = tree_weights
    return consts


_NC_CACHE = {}


def _build_bass():
    import concourse.bacc as bacc
    import concourse.mybir as mybir
    import concourse.tile as tile
    from concourse.hw_specs import get_activation_tables
    from concourse.masks import make_identity

    dt = mybir.dt
    f32 = dt.float32
    f32r = dt.float32r
    bf16 = dt.bfloat16
    fp16 = dt.float16
    AF = mybir.ActivationFunctionType
    ALU = mybir.AluOpType
    AX = mybir.AxisListType

    nc = bacc.Bacc("TRN2", target_bir_lowering=False, debug=False,
                   num_devices=NCORES)

    # Pin the ACT function table to one containing BOTH Exp and Ln, else the
    # table-load pass ping-pongs between single-function tables (~1.3us per
    # reload, one per activation).
    table_id = next(i for i, (_, funcs) in
                    enumerate(get_activation_tables("gen3").items())
                    if AF.Exp in funcs and AF.Ln in funcs)
    nc.scalar.add_instruction(mybir.InstLoadActFuncSet(
        name=f"I-{nc.next_id()}", ins=[], outs=[], act_func_set_id=table_id))

    xt = nc.dram_tensor("xt", [D, BS], fp16, kind="ExternalInput").ap()
    wt = nc.dram_tensor("wt", [D, TNP], fp16, kind="ExternalInput").ap()
    consts = nc.dram_tensor("consts", [128, _CONST_COLS], f32r,
                            kind="ExternalInput").ap()
    amat = nc.dram_tensor("amat", [128, 256], fp16, kind="ExternalInput").ap()
    llf = nc.dram_tensor("llf", [TNP, C], fp16, kind="ExternalInput").ap()
    out = nc.dram_tensor("out", [C, BS], f32, kind="ExternalOutput").ap()

    with tile.TileContext(nc) as tc:
        with (
            tc.tile_pool(name="big", bufs=1) as bigp,
            tc.tile_pool(name="const", bufs=1) as constp,
            tc.tile_pool(name="work", bufs=3) as work,
            tc.tile_pool(name="tmp", bufs=2) as tmpp,
            tc.tile_pool(name="ps", bufs=2, space="PSUM") as psp,
            tc.tile_pool(name="ps1", bufs=1, space="PSUM") as ps1,
        ):
            # ---- input loads, ordered for earliest PE start -----------
            wt_t = [bigp.tile([128, TNP], fp16, tag=f"wt{j}", name=f"wt{j}")
                    for j in range(4)]
            xt_t = bigp.tile([128, 4 * BS], fp16, tag="xt")
            consts_t = constp.tile([128, _CONST_COLS], f32r, tag="consts")
            ll_t = bigp.tile([128, NTILES * C], fp16, tag="ll")
            ident = constp.tile([64, 64], f32, tag="ident")
            make_identity(nc, ident[:])

            CH = TNP // 4
            nc.scalar.dma_start(out=wt_t[0][:, 0:CH], in_=wt[0:128, 0:CH])
            nc.sync.dma_start(
                out=xt_t[:].rearrange("p (j b) -> p j b", b=BS),
                in_=xt.rearrange("(j p) b -> p j b", p=128),
            )
            nc.scalar.dma_start(out=wt_t[1][:, 0:CH], in_=wt[128:256, 0:CH])
            nc.sync.dma_start(out=wt_t[2][:, 0:CH], in_=wt[256:384, 0:CH])
            nc.scalar.dma_start(out=wt_t[3][:, 0:CH], in_=wt[384:512, 0:CH])
            nc.sync.dma_start(out=consts_t[:], in_=consts[:])
            engs3 = [nc.sync, nc.scalar, nc.gpsimd]
            for q in range(1, 4):
                cs, ce = q * CH, (q + 1) * CH
                for j in range(4):
                    eng = engs3[(4 * q + j) % 3]
                    eng.dma_start(out=wt_t[j][:, cs:ce],
                                  in_=wt[j * 128:(j + 1) * 128, cs:ce])
            nc.scalar.dma_start(
                out=ll_t[:].rearrange("p (i c) -> p i c", c=C),
                in_=llf.rearrange("(i p) c -> p i c", p=128),
            )

            amat_t = constp.tile([128, 256], fp16, tag="amat")
            nc.sync.dma_start(out=amat_t[:], in_=amat[:])
            adir_ap = amat_t[:, 0:128]
            apath_ap = amat_t[:, 128:256]
            a2_ap = consts_t[0:T, _COL_A2:_COL_A2 + 128].bitcast(f32)
            e2_ap = consts_t[0:T, _COL_E2:_COL_E2 + NTILES].bitcast(f32)
            tw_ap = consts_t[0:1, _COL_TW:_COL_TW + T].bitcast(f32)

            def bias_ap(i):
                return consts_t[:, _COL_BIAS + i:_COL_BIAS + i + 1].bitcast(f32)

            # ---- main pipeline (two 128-tiles per step) --------------
            out_ps = ps1.tile([C, BS], f32, tag="outps")
            ta2 = tb2 = None
            pending_tail = None
            for i in range(NTILES):
                pz = psp.tile([128, BS], f32, tag="pz")
                for j in range(4):
                    nc.tensor.matmul(
                        pz[:],
                        lhsT=wt_t[j][:, i * 128:(i + 1) * 128],
                        rhs=xt_t[:, j * BS:(j + 1) * BS],
                        start=(j == 0), stop=(j == 3),
                    )
                if i == 0:
                    # ---- tree-weight softmax -> per-partition scale columns ----
                    mneg = constp.tile([1, 1], f32, tag="mneg")
                    nc.vector.tensor_reduce(out=mneg[:], in_=tw_ap, op=ALU.max,
                                            axis=AX.X, negate=True)
                    ew = constp.tile([1, T], f32, tag="ew")
                    nc.scalar.activation(ew[:], tw_ap, AF.Exp, bias=mneg[:, 0:1],
                                         scale=1.0)
                    sw = constp.tile([1, 1], f32, tag="sw")
                    nc.vector.tensor_reduce(out=sw[:], in_=ew[:], op=ALU.add, axis=AX.X)
                    rw = constp.tile([1, 1], f32, tag="rw")
                    nc.vector.reciprocal(rw[:], sw[:])
                    wrow = constp.tile([1, T], f32, tag="wrow")
                    nc.vector.tensor_scalar(out=wrow[:], in0=ew[:], scalar1=rw[:, 0:1],
                                            scalar2=2.0, op0=ALU.mult, op1=ALU.mult)
                    wcol_ps = ps1.tile([T, 1], f32, tag="early")
                    nc.tensor.transpose(wcol_ps[:], wrow[:], ident[0:1, 0:1])
                    wcol = constp.tile([T, 1], f32, tag="wcol")
                    nc.vector.tensor_copy(out=wcol[:], in_=wcol_ps[:])
                    bmat = constp.tile([T, NTILES], f32, tag="bmat")
                    nc.vector.tensor_scalar_mul(bmat[:], e2_ap, wcol[:, 0:1])
                    w2_ps = ps1.tile([128, NTILES], f32, tag="early")
                    nc.tensor.matmul(w2_ps[:], lhsT=a2_ap, rhs=bmat[:],
                                     start=True, stop=True)
                    w2c = constp.tile([128, NTILES], f32, tag="w2c")
                    nc.vector.tensor_copy(out=w2c[:], in_=w2_ps[:])

                    # ---- leaf distributions: one big exp + rowsum ------------
                    ev_all = bigp.tile([128, NTILES * C], f32, tag="evall")
                    nc.scalar.activation(ev_all[:], ll_t[:], AF.Exp)
                    sv_all = constp.tile([128, NTILES], f32, tag="svall")
                    nc.vector.tensor_reduce(
                        out=sv_all[:],
                        in_=ev_all[:].rearrange("p (i c) -> p i c", c=C),
                        op=ALU.add, axis=AX.X)
                    rv_all = constp.tile([128, NTILES], f32, tag="rvall")
                    nc.vector.reciprocal(rv_all[:], sv_all[:])
                    rw2 = constp.tile([128, NTILES], f32, tag="rw2")
                    nc.vector.tensor_tensor(out=rw2[:], in0=rv_all[:],
                                            in1=w2c[:], op=ALU.mult)
                    vt_all = bigp.tile([128, NTILES * C], fp16, tag="vtall")
                    nc.vector.tensor_tensor(
                        out=vt_all[:].rearrange("p (i c) -> p i c", c=C),
                        in0=ev_all[:].rearrange("p (i c) -> p i c", c=C),
                        in1=rw2[:].unsqueeze(2).broadcast_to(
                            [128, NTILES, C]),
                        op=ALU.mult)


                if i % 4 == 0:
                    ta2 = work.tile([128, 4 * BS], fp16, tag="ta2")
                    tb2 = work.tile([128, 4 * BS], fp16, tag="tb2")
                half = (i % 4) * BS
                ta = ta2[:, half:half + BS]
                nc.vector.tensor_scalar_add(out=ta, in0=pz[:],
                                            scalar1=bias_ap(i))
                if i % 4 == 3:
                    # one Exp + one Ln covering four tiles (SBUF source)
                    te = tmpp.tile([128, 4 * BS], f32, tag="te")
                    nc.scalar.activation(te[:], ta2[:], AF.Exp)
                    nc.scalar.activation(tb2[:], te[:], AF.Ln, bias=1.0,
                                         scale=1.0)

                    def quad_tail(i=i, ta2=ta2, tb2=tb2):
                        for g in range(2):
                            pp = psp.tile([128, 2 * BS], f32, tag="pp",
                                          name="pp")
                            for h2 in range(2):
                                h = 2 * g + h2
                                sl = slice(h2 * BS, (h2 + 1) * BS)
                                nc.tensor.matmul(
                                    pp[:, sl], lhsT=adir_ap,
                                    rhs=ta2[:, h * BS:(h + 1) * BS],
                                    start=True, stop=False)
                                nc.tensor.matmul(
                                    pp[:, sl], lhsT=apath_ap,
                                    rhs=tb2[:, h * BS:(h + 1) * BS],
                                    start=False, stop=True)
                            lp = work.tile([128, 2 * BS], fp16, tag="lp",
                                           name="lp")
                            nc.scalar.activation(lp[:], pp[:], AF.Exp)
                            for h2 in range(2):
                                ii = i - 3 + 2 * g + h2
                                nc.tensor.matmul(
                                    out_ps[:],
                                    lhsT=vt_all[:, ii * C:(ii + 1) * C],
                                    rhs=lp[:, h2 * BS:(h2 + 1) * BS],
                                    start=(ii == 0),
                                    stop=(ii == NTILES - 1))

                    # defer this quad's path-sum/output matmuls until after
                    # the NEXT quad's stage-1 block, so the PE stream never
                    # waits on this quad's Ln.
                    if pending_tail is not None:
                        pending_tail()
                    pending_tail = quad_tail

            if pending_tail is not None:
                pending_tail()

            out_sb = work.tile([C, BS], f32, tag="osb")
            nc.vector.tensor_copy(out=out_sb[:], in_=out_ps[:])
            nc.sync.dma_start(out=out[:], in_=out_sb[:])

    nc.finalize()
    return nc


def _get_nc():
    if "nc" not in _NC_CACHE:
        _NC_CACHE["nc"] = _build_bass()
    return _NC_CACHE["nc"]


def _prep_inputs(x, split_weights, split_bias, leaf_logits, tree_weights):
    import ml_dtypes

    x = np.asarray(x, np.float32)
    split_weights = np.asarray(split_weights, np.float32)
    split_bias = np.asarray(split_bias, np.float32)
    leaf_logits = np.asarray(leaf_logits, np.float32)
    tree_weights = np.asarray(tree_weights, np.float32)

    wpad = np.zeros((T, NPAD, D), np.float32)
    wpad[:, :N, :] = split_weights
    wtT = np.ascontiguousarray(
        wpad.reshape(TNP, D).T.astype(np.float16))              # [D, TNP]
    consts = _pack_consts(split_bias, tree_weights)
    llf = np.ascontiguousarray(
        leaf_logits.reshape(TNP, C).astype(np.float16))

    idx, dr = _leaf_paths(TREE_DEPTH)
    amat = np.zeros((128, 256), np.float16)
    amat[:, 0:128] = consts[:, _COL_ADIR:_COL_ADIR + 128]
    amat[:, 128:256] = consts[:, _COL_APATH:_COL_APATH + 128]
    shared = dict(wt=wtT, consts=consts, llf=llf, amat=amat)
    in_maps = []
    for i in range(NCORES):
        xti = np.ascontiguousarray(
            x[i * BS:(i + 1) * BS, :].T.astype(np.float16))          # [D, BS]
        in_maps.append(dict(xt=xti, **shared))
    return in_maps


def kernel(x, split_weights, split_bias, leaf_logits, tree_weights):
    from concourse.bass_utils import run_bass_kernel_spmd

    in_maps = _prep_inputs(x, split_weights, split_bias, leaf_logits,
                           tree_weights)
    nc = _get_nc()
    res = run_bass_kernel_spmd(nc, in_maps, core_ids=list(range(NCORES)))
    out = np.concatenate([res.results[i]["out"] for i in range(NCORES)],
                         axis=1).T                              # [B, C]
    return np.ascontiguousarray(out.astype(np.float32))

